# revision 12
# baseline (speedup 1.0000x reference)
"""Trainium2 Bass kernel for nn_Airnet (gated RNN scanned over batch dim).

Key algebraic reduction: the reference scans over the leading (batch) dim with
state h of shape [T, H], but every op in the step function is row-wise over T
and only h[-1] (row T-1 = 511) ever feeds the output head.  The T rows evolve
independently, so the whole computation reduces exactly to a single-row
recurrence:

    x_b   = inputs[b, T-1, :]                          (B=256 steps)
    xp_b  = Wih @ x_b + Bih                            (precomputable, parallel)
    hp    = Whh @ h + Bhh                              (sequential matvec)
    fG    = sigmoid(xp_b[:H] + hp[:H])
    hG    = tanh(xp_b[H:] + fG * hp[H:])
    h     = (1-fG) * h + fG * hG ;  lasts[b] = h
    out   = lasts @ Wout.T + Bout

Device mapping (the recurrence is strictly sequential and fits one core, so
all 8 cores run identical replicas and core 0's output is returned):

  - XP precompute: 16 psum tiles [128, 256] via fp32r matmuls (Wih^T
    stationary, x^T streamed), bias added as per-partition scalars.
  - Per step: 32 fp32r matmuls stream Whh^T (h column stationary,
    [K=128,M=1] x [K=128,N=512]) accumulating hp^T into 4 psum chunks
    [1, 512]; VectorE evacuates each chunk to an SBUF row; 16 tiny PE
    transposes ([1,128] -> [128,1]) land hp as [128, 16] in psum;
    VectorE/ScalarE compute the gates in [128, 8] layout and write h
    directly into its slot of `lasts` (which is also the matmul lhsT
    source for the next step).
  - Head: fp32 matmuls over lasts columns.

Scheduling constraints honored (walrus wait-count limits): every matmul /
DMA may carry at most ONE unobserved semaphore, so all matmul and gate
operands are produced by VectorE ops (casts/copies), never consumed straight
from a DMA; fp32r operands are produced by explicit DVE casts (in-place via
bitcast views) because the BIR verifier requires fp32r inputs to be rounded.

Layouts (hidden unit u = 128*kc + p lives at partition p, column kc):
  whhT[p, kc, g] = Whh[g, 128*kc+p];  xT[p, kc, b] = inputs[b, -1, 128*kc+p]
  wihT[p, kc, g] = Wih[g, 128*kc+p];  woutT[p, j, o] = Wout[o, 128*j+p]
  XPT[p, b, j]   = (x_b @ Wih.T)[128*j+p] + bias1c[p, j]
"""
import numpy as np

import concourse.bass as bass
import concourse.tile as tile
from concourse import bacc, mybir
from concourse.bass_utils import run_bass_kernel_spmd

F32 = mybir.dt.float32
F32R = mybir.dt.float32r

B, T, I, H, O = 256, 512, 256, 1024, 128
G = 2 * H
NCORES = 8
STEPS = B


def build(steps=STEPS):
    nc = bacc.Bacc("TRN2", target_bir_lowering=False, debug=False)
    xT_d = nc.declare_dram_parameter("xT", [128, 2, B], F32, isOutput=False)
    wihT_d = nc.declare_dram_parameter("wihT", [128, 2, G], F32, isOutput=False)
    whhT_d = nc.declare_dram_parameter("whhT", [128, 8, G], F32, isOutput=False)
    bias1c_d = nc.declare_dram_parameter("bias1c", [128, 16], F32, isOutput=False)
    bhhH_d = nc.declare_dram_parameter("bhhH", [128, 8], F32, isOutput=False)
    woutT_d = nc.declare_dram_parameter("woutT", [128, 8, O], F32, isOutput=False)
    bout_d = nc.declare_dram_parameter("bout", [1, O], F32, isOutput=False)
    out_d = nc.declare_dram_parameter("out", [B, O], F32, isOutput=True)

    with tile.TileContext(nc) as tc:
        with (
            tc.tile_pool(name="persist", bufs=1) as pp,
            tc.tile_pool(name="work", bufs=2) as wp,
            tc.tile_pool(name="row", bufs=1) as rp,
            tc.tile_pool(name="psum", bufs=1, space="PSUM") as ps,
        ):
            whhTr = pp.tile([128, 8, G], F32R)
            XPT = pp.tile([128, B, 16], F32)
            lasts = pp.tile([128, B + 1, 8], F32)
            bhhH = pp.tile([128, 8], F32)
            bias1c = pp.tile([128, 16], F32)
            ones = pp.tile([1, 128], F32)
            woutT = pp.tile([128, 8, O], F32)
            bout2 = pp.tile([1, O], F32)

            # --- loads; every matmul/gate operand is (re)produced on DVE
            # (out-of-place fp32->fp32r casts: the BIR verifier requires
            # fp32r matmul inputs to be produced by a rounding op).  All
            # staging lives in ONE transient pool so no later allocation
            # reuses freed space (avoids multi-wait DMA edges). ---
            lp_cm = tc.tile_pool(name="load", bufs=1)
            lp = lp_cm.__enter__()
            whh_stage = lp.tile([128, 8, G], F32)
            nc.sync.dma_start(whh_stage[:], whhT_d[:])
            nc.vector.tensor_copy(whhTr[:], whh_stage[:])

            nc.sync.dma_start(bhhH[:], bhhH_d[:])
            nc.sync.dma_start(bias1c[:], bias1c_d[:])
            nc.sync.dma_start(woutT[:], woutT_d[:])
            nc.sync.dma_start(bout2[:], bout_d[:])
            nc.vector.memset(ones[:], 1.0)
            nc.vector.memset(lasts[:, 0, :], 0.0)

            # ---------------- XP precompute (fp32r) ----------------
            with nc.named_scope("xp"):
                wihT0 = lp.tile([128, 2, G], F32)
                xTt0 = lp.tile([128, 2, B], F32)
                nc.sync.dma_start(wihT0[:], wihT_d[:])
                nc.sync.dma_start(xTt0[:], xT_d[:])
                for j in range(16):
                    q = ps.tile([128, B], F32, tag="q")
                    for kc in range(2):
                        nc.tensor.matmul(
                            q[:],
                            wihT0[:, kc, 128 * j : 128 * (j + 1)],
                            xTt0[:, kc, :],
                            start=(kc == 0),
                            stop=(kc == 1),
                        )
                    nc.vector.tensor_copy(XPT[:, :, j], q[:])
                for j in range(16):
                    nc.vector.tensor_scalar_add(
                        XPT[:, :, j], XPT[:, :, j], bias1c[:, j : j + 1]
                    )
            lp_cm.__exit__(None, None, None)

            # ---------------- recurrence ----------------
            with nc.named_scope("loop"):
                for b in range(steps):
                    hcols = lasts[:, b, :]
                    hcR = wp.tile([128, 8], F32R, tag="hcR")
                    nc.vector.tensor_copy(hcR[:], hcols)
                    rowA = rp.tile([1, G], F32, tag="rowA")
                    for c in range(4):
                        pc = ps.tile([1, 512], F32, tag=f"pc{c}")
                        for kc in range(8):
                            nc.tensor.matmul(
                                pc[:],
                                hcR[:, kc : kc + 1],
                                whhTr[:, kc, 512 * c : 512 * (c + 1)],
                                start=(kc == 0),
                                stop=(kc == 7),
                            )
                        nc.vector.tensor_copy(
                            rowA[0:1, 512 * c : 512 * (c + 1)], pc[:]
                        )
                    # 16x PE transpose [1,128] -> [128,1]: hpT[p, j] = hp[128j+p]
                    hpT = ps.tile([128, 16], F32, tag="hpT")
                    for j in range(16):
                        nc.tensor.transpose(
                            hpT[:, j : j + 1],
                            rowA[0:1, 128 * j : 128 * (j + 1)],
                            ones[0:1, 0:1],
                        )
                    # gates
                    af = wp.tile([128, 8], F32, tag="af")
                    fG = wp.tile([128, 8], F32, tag="fG")
                    hh2 = wp.tile([128, 8], F32, tag="hh2")
                    ah = wp.tile([128, 8], F32, tag="ah")
                    hG = wp.tile([128, 8], F32, tag="hG")
                    dd = wp.tile([128, 8], F32, tag="dd")
                    nc.vector.tensor_add(af[:], hpT[:, 0:8], XPT[:, b, 0:8])
                    nc.scalar.activation(
                        fG[:], af[:], mybir.ActivationFunctionType.Sigmoid
                    )
                    nc.vector.tensor_add(hh2[:], hpT[:, 8:16], bhhH[:])
                    nc.vector.tensor_mul(ah[:], fG[:], hh2[:])
                    nc.vector.tensor_add(ah[:], ah[:], XPT[:, b, 8:16])
                    nc.scalar.activation(
                        hG[:], ah[:], mybir.ActivationFunctionType.Tanh
                    )
                    nc.vector.tensor_sub(dd[:], hG[:], hcols)
                    nc.vector.tensor_mul(dd[:], fG[:], dd[:])
                    nc.vector.tensor_add(lasts[:, b + 1, :], hcols, dd[:])

            # ---------------- head (fp32) ----------------
            with nc.named_scope("head"):
                for mb in range(B // 128):
                    ph = ps.tile([128, O], F32, tag="ho")
                    for j in range(8):
                        nc.tensor.matmul(
                            ph[:],
                            lasts[:, 1 + 128 * mb : 1 + 128 * (mb + 1), j],
                            woutT[:, j, :],
                            start=(j == 0),
                            stop=False,
                        )
                    nc.tensor.matmul(
                        ph[:],
                        ones[0:1, 0:128],
                        bout2[0:1, :],
                        start=False,
                        stop=True,
                    )
                    outS = wp.tile([128, O], F32, tag="outS")
                    nc.vector.tensor_copy(outS[:], ph[:])
                    nc.sync.dma_start(out_d[128 * mb : 128 * (mb + 1), :], outS[:])
    nc.compile()
    return nc


def prep_inputs(inputs, Wih, Whh, Bih, Bhh, Wout, Bout):
    inputs = np.asarray(inputs, np.float32)
    Wih = np.asarray(Wih, np.float32)
    Whh = np.asarray(Whh, np.float32)
    Bih = np.asarray(Bih, np.float32)
    Bhh = np.asarray(Bhh, np.float32)
    Wout = np.asarray(Wout, np.float32)
    Bout = np.asarray(Bout, np.float32)
    x = inputs[:, T - 1, :]  # [B, I] — only row T-1 feeds the output
    xT = np.ascontiguousarray(x.reshape(B, 2, 128).transpose(2, 1, 0))
    wihT = np.ascontiguousarray(Wih.reshape(G, 2, 128).transpose(2, 1, 0))
    whhT = np.ascontiguousarray(Whh.reshape(G, 8, 128).transpose(2, 1, 0))
    bias1 = Bih + np.concatenate([Bhh[:H], np.zeros(H, np.float32)])
    bias1c = np.ascontiguousarray(bias1.reshape(16, 128).T)
    bhhH = np.ascontiguousarray(Bhh[H:].reshape(8, 128).T)
    woutT = np.ascontiguousarray(Wout.reshape(O, 8, 128).transpose(2, 1, 0))
    return {
        "xT": xT,
        "wihT": wihT,
        "whhT": whhT,
        "bias1c": bias1c,
        "bhhH": bhhH,
        "woutT": woutT,
        "bout": np.ascontiguousarray(Bout[None, :], np.float32),
    }


def run(inputs, Wih, Whh, Bih, Bhh, Wout, Bout, trace=False, ncores=NCORES):
    ins = prep_inputs(inputs, Wih, Whh, Bih, Bhh, Wout, Bout)
    nc = build()
    # Only core 0 gets the real inputs; the other replicas get zero-filled
    # buffers (zstd-compressed to ~nothing on the wire) since their outputs
    # are discarded.
    zins = {k: np.zeros_like(v) for k, v in ins.items()}
    in_maps = [dict(ins)] + [dict(zins) for _ in range(ncores - 1)]
    r = run_bass_kernel_spmd(nc, in_maps, core_ids=list(range(ncores)), trace=trace)
    return np.asarray(r.results[0]["out"], np.float32), r


def kernel(inputs, Wih, Whh, Bih, Bhh, Wout, Bout):
    out, _ = run(inputs, Wih, Whh, Bih, Bhh, Wout, Bout)
    return out


# revision 13
# speedup vs baseline: 1.1565x; 1.1565x over previous
"""Trainium2 Bass kernel for nn_Airnet (gated RNN scanned over batch dim).

Key algebraic reduction: the reference scans over the leading (batch) dim with
state h of shape [T, H], but every op in the step function is row-wise over T
and only h[-1] (row T-1 = 511) ever feeds the output head.  The T rows evolve
independently, so the whole computation reduces exactly to a single-row
recurrence:

    x_b   = inputs[b, T-1, :]                          (B=256 steps)
    xp_b  = Wih @ x_b + Bih                            (precomputable, parallel)
    hp    = Whh @ h + Bhh                              (sequential matvec)
    fG    = sigmoid(xp_b[:H] + hp[:H])
    hG    = tanh(xp_b[H:] + fG * hp[H:])
    h     = (1-fG) * h + fG * hG ;  lasts[b] = h
    out   = lasts @ Wout.T + Bout

Device mapping (the recurrence is strictly sequential and fits one core, so
all 8 cores run identical replicas and core 0's output is returned):

  - XP precompute: 16 psum tiles [128, 256] via fp32r matmuls (Wih^T
    stationary, x^T streamed), bias added as per-partition scalars.
  - Per step: 32 fp32r matmuls stream Whh^T (h column stationary,
    [K=128,M=1] x [K=128,N=512]) accumulating hp^T into 4 psum chunks
    [1, 512]; VectorE evacuates each chunk to an SBUF row; 16 tiny PE
    transposes ([1,128] -> [128,1]) land hp as [128, 16] in psum;
    VectorE/ScalarE compute the gates in [128, 8] layout and write h
    directly into its slot of `lasts` (which is also the matmul lhsT
    source for the next step).
  - Head: fp32 matmuls over lasts columns.

Scheduling constraints honored (walrus wait-count limits): every matmul /
DMA may carry at most ONE unobserved semaphore, so all matmul and gate
operands are produced by VectorE ops (casts/copies), never consumed straight
from a DMA; fp32r operands are produced by explicit DVE casts (in-place via
bitcast views) because the BIR verifier requires fp32r inputs to be rounded.

Layouts (hidden unit u = 128*kc + p lives at partition p, column kc):
  whhT[p, kc, g] = Whh[g, 128*kc+p];  xT[p, kc, b] = inputs[b, -1, 128*kc+p]
  wihT[p, kc, g] = Wih[g, 128*kc+p];  woutT[p, j, o] = Wout[o, 128*j+p]
  XPT[p, b, j]   = (x_b @ Wih.T)[128*j+p] + bias1c[p, j]
"""
import numpy as np

import concourse.bass as bass
import concourse.tile as tile
from concourse import bacc, mybir
from concourse.bass_utils import run_bass_kernel_spmd

F32 = mybir.dt.float32
F32R = mybir.dt.float32r

B, T, I, H, O = 256, 512, 256, 1024, 128
G = 2 * H
NCORES = 8
STEPS = B


def build(steps=STEPS):
    nc = bacc.Bacc("TRN2", target_bir_lowering=False, debug=False)
    xT_d = nc.declare_dram_parameter("xT", [128, 2, B], F32, isOutput=False)
    wihT_d = nc.declare_dram_parameter("wihT", [128, 2, G], F32, isOutput=False)
    whhT_d = nc.declare_dram_parameter("whhT", [128, 8, G], F32, isOutput=False)
    bias1c_d = nc.declare_dram_parameter("bias1c", [128, 16], F32, isOutput=False)
    bhhH_d = nc.declare_dram_parameter("bhhH", [128, 8], F32, isOutput=False)
    woutT_d = nc.declare_dram_parameter("woutT", [128, 8, O], F32, isOutput=False)
    bout_d = nc.declare_dram_parameter("bout", [1, O], F32, isOutput=False)
    out_d = nc.declare_dram_parameter("out", [B, O], F32, isOutput=True)

    with tile.TileContext(nc) as tc:
        with (
            tc.tile_pool(name="persist", bufs=1) as pp,
            tc.tile_pool(name="work", bufs=2) as wp,
            tc.tile_pool(name="row", bufs=1) as rp,
            tc.tile_pool(name="psum", bufs=1, space="PSUM") as ps,
        ):
            whhTr = pp.tile([128, 8, G], F32R)
            XPT = pp.tile([128, B, 16], F32)
            lasts = pp.tile([128, B + 1, 8], F32)
            bhhH = pp.tile([128, 8], F32)
            bias1c = pp.tile([128, 16], F32)
            ones = pp.tile([1, 128], F32)
            woutT = pp.tile([128, 8, O], F32)
            bout2 = pp.tile([1, O], F32)

            # --- loads; every matmul/gate operand is (re)produced on DVE
            # (out-of-place fp32->fp32r casts: the BIR verifier requires
            # fp32r matmul inputs to be produced by a rounding op).  All
            # staging lives in ONE transient pool so no later allocation
            # reuses freed space (avoids multi-wait DMA edges). ---
            lp_cm = tc.tile_pool(name="load", bufs=1)
            lp = lp_cm.__enter__()
            whh_stage = lp.tile([128, 8, G], F32)
            nc.sync.dma_start(whh_stage[:], whhT_d[:])
            nc.vector.tensor_copy(whhTr[:], whh_stage[:])

            nc.sync.dma_start(bhhH[:], bhhH_d[:])
            nc.sync.dma_start(bias1c[:], bias1c_d[:])
            nc.sync.dma_start(woutT[:], woutT_d[:])
            nc.sync.dma_start(bout2[:], bout_d[:])
            nc.vector.memset(ones[:], 1.0)
            nc.vector.memset(lasts[:, 0, :], 0.0)

            # ---------------- XP precompute (fp32r) ----------------
            with nc.named_scope("xp"):
                wihT0 = lp.tile([128, 2, G], F32)
                xTt0 = lp.tile([128, 2, B], F32)
                nc.sync.dma_start(wihT0[:], wihT_d[:])
                nc.sync.dma_start(xTt0[:], xT_d[:])
                for j in range(16):
                    q = ps.tile([128, B], F32, tag="q")
                    for kc in range(2):
                        nc.tensor.matmul(
                            q[:],
                            wihT0[:, kc, 128 * j : 128 * (j + 1)],
                            xTt0[:, kc, :],
                            start=(kc == 0),
                            stop=(kc == 1),
                        )
                    nc.vector.tensor_copy(XPT[:, :, j], q[:])
                for j in range(16):
                    nc.vector.tensor_scalar_add(
                        XPT[:, :, j], XPT[:, :, j], bias1c[:, j : j + 1]
                    )
            lp_cm.__exit__(None, None, None)

            # ---------------- recurrence ----------------
            # Two half-pipelines per step: chunks (0,2) cover units 0-511
            # (f-gates j0-3 + h-gates j8-11), chunks (1,3) cover units
            # 512-1023.  Half A's transposes/gates/h-write overlap half B's
            # matmul stream, and the next step's kc0-3 matmuls can start as
            # soon as half A's h columns land.
            with nc.named_scope("loop"):
                for b in range(steps):
                    halves = (
                        # (chunks, hcR cols, lasts cols, XPT f, XPT h, bhh cols)
                        ((0, 2), 0),
                        ((1, 3), 1),
                    )
                    hcRs = []
                    for hv in range(2):
                        hcR = wp.tile([128, 4], F32R, tag=f"hcR{hv}")
                        nc.vector.tensor_copy(hcR[:], lasts[:, b, 4 * hv : 4 * (hv + 1)])
                        hcRs.append(hcR)
                    rowA = rp.tile([1, G], F32, tag="rowA")
                    for chunks, hv in halves:
                        hpT = ps.tile([128, 8], F32, tag=f"hpT{hv}")
                        for ci, c in enumerate(chunks):
                            pc = ps.tile([1, 512], F32, tag=f"pc{c}")
                            for kc in range(8):
                                nc.tensor.matmul(
                                    pc[:],
                                    hcRs[kc // 4][:, kc % 4 : kc % 4 + 1],
                                    whhTr[:, kc, 512 * c : 512 * (c + 1)],
                                    start=(kc == 0),
                                    stop=(kc == 7),
                                )
                            nc.vector.tensor_copy(
                                rowA[0:1, 512 * c : 512 * (c + 1)], pc[:]
                            )
                            # transpose this chunk's 4 columns into hpT half
                            for jj in range(4):
                                j = 4 * c + jj
                                nc.tensor.transpose(
                                    hpT[:, 4 * ci + jj : 4 * ci + jj + 1],
                                    rowA[0:1, 128 * j : 128 * (j + 1)],
                                    ones[0:1, 0:1],
                                )
                        # gates for this half's 512 units ([128, 4] tiles)
                        lo, hi = 4 * hv, 4 * (hv + 1)
                        af = wp.tile([128, 4], F32, tag=f"af{hv}")
                        fG = wp.tile([128, 4], F32, tag=f"fG{hv}")
                        hh2 = wp.tile([128, 4], F32, tag=f"hh2{hv}")
                        ah = wp.tile([128, 4], F32, tag=f"ah{hv}")
                        hG = wp.tile([128, 4], F32, tag=f"hG{hv}")
                        dd = wp.tile([128, 4], F32, tag=f"dd{hv}")
                        nc.vector.tensor_add(af[:], hpT[:, 0:4], XPT[:, b, lo:hi])
                        nc.scalar.activation(
                            fG[:], af[:], mybir.ActivationFunctionType.Sigmoid
                        )
                        nc.vector.tensor_add(hh2[:], hpT[:, 4:8], bhhH[:, lo:hi])
                        nc.vector.tensor_mul(ah[:], fG[:], hh2[:])
                        nc.vector.tensor_add(ah[:], ah[:], XPT[:, b, 8 + lo : 8 + hi])
                        nc.scalar.activation(
                            hG[:], ah[:], mybir.ActivationFunctionType.Tanh
                        )
                        nc.vector.tensor_sub(dd[:], hG[:], lasts[:, b, lo:hi])
                        nc.vector.tensor_mul(dd[:], fG[:], dd[:])
                        nc.vector.tensor_add(
                            lasts[:, b + 1, lo:hi], lasts[:, b, lo:hi], dd[:]
                        )

            # ---------------- head (fp32) ----------------
            with nc.named_scope("head"):
                for mb in range(B // 128):
                    ph = ps.tile([128, O], F32, tag="ho")
                    for j in range(8):
                        nc.tensor.matmul(
                            ph[:],
                            lasts[:, 1 + 128 * mb : 1 + 128 * (mb + 1), j],
                            woutT[:, j, :],
                            start=(j == 0),
                            stop=False,
                        )
                    nc.tensor.matmul(
                        ph[:],
                        ones[0:1, 0:128],
                        bout2[0:1, :],
                        start=False,
                        stop=True,
                    )
                    outS = wp.tile([128, O], F32, tag="outS")
                    nc.vector.tensor_copy(outS[:], ph[:])
                    nc.sync.dma_start(out_d[128 * mb : 128 * (mb + 1), :], outS[:])
    nc.compile()
    return nc


def prep_inputs(inputs, Wih, Whh, Bih, Bhh, Wout, Bout):
    inputs = np.asarray(inputs, np.float32)
    Wih = np.asarray(Wih, np.float32)
    Whh = np.asarray(Whh, np.float32)
    Bih = np.asarray(Bih, np.float32)
    Bhh = np.asarray(Bhh, np.float32)
    Wout = np.asarray(Wout, np.float32)
    Bout = np.asarray(Bout, np.float32)
    x = inputs[:, T - 1, :]  # [B, I] — only row T-1 feeds the output
    xT = np.ascontiguousarray(x.reshape(B, 2, 128).transpose(2, 1, 0))
    wihT = np.ascontiguousarray(Wih.reshape(G, 2, 128).transpose(2, 1, 0))
    whhT = np.ascontiguousarray(Whh.reshape(G, 8, 128).transpose(2, 1, 0))
    bias1 = Bih + np.concatenate([Bhh[:H], np.zeros(H, np.float32)])
    bias1c = np.ascontiguousarray(bias1.reshape(16, 128).T)
    bhhH = np.ascontiguousarray(Bhh[H:].reshape(8, 128).T)
    woutT = np.ascontiguousarray(Wout.reshape(O, 8, 128).transpose(2, 1, 0))
    return {
        "xT": xT,
        "wihT": wihT,
        "whhT": whhT,
        "bias1c": bias1c,
        "bhhH": bhhH,
        "woutT": woutT,
        "bout": np.ascontiguousarray(Bout[None, :], np.float32),
    }


def run(inputs, Wih, Whh, Bih, Bhh, Wout, Bout, trace=False, ncores=NCORES):
    ins = prep_inputs(inputs, Wih, Whh, Bih, Bhh, Wout, Bout)
    nc = build()
    # Only core 0 gets the real inputs; the other replicas get zero-filled
    # buffers (zstd-compressed to ~nothing on the wire) since their outputs
    # are discarded.
    zins = {k: np.zeros_like(v) for k, v in ins.items()}
    in_maps = [dict(ins)] + [dict(zins) for _ in range(ncores - 1)]
    r = run_bass_kernel_spmd(nc, in_maps, core_ids=list(range(ncores)), trace=trace)
    return np.asarray(r.results[0]["out"], np.float32), r


def kernel(inputs, Wih, Whh, Bih, Bhh, Wout, Bout):
    out, _ = run(inputs, Wih, Whh, Bih, Bhh, Wout, Bout)
    return out
